# revision 14
# baseline (speedup 1.0000x reference)
"""MoE-LoRA fused attention kernel for 8 Trainium2 NeuronCores.

Problem: x[8,512,768] -> qkv = x@W_qkv.T + top2-routed LoRA experts;
multi-head attention (12 heads, hd=64); out-projection.

Sharding: data-parallel over batch. Core b handles batch element b
(attention + routing are token-local, so there is no cross-core
communication at all).

Per-core layout strategy (everything host-pre-transposed so the device
does no transposes on the forward path):
  xT      [768, 512]   (d on partitions, t free)
  q,k     computed transposed:  qkT[o, t] = sum_d W[o,d] xT[d,t]
  v       computed natural:     v[t, o]
  scores  computed transposed:  st[kt, q] = kT.T @ qT  (exp is elementwise;
          the softmax normalizer Z[q] = sum_k exp(st) falls out of the
          O-matmul as a ones-column appended to v)
  O       computed natural:     O[q, hd|Z] = st_exp.T @ [v | 1]
  proj    needs attn_out transposed -> 24 PE transposes, then
          final[t, o] = attn_outT.T @ W_projT
Matmuls run as float32r (full PE rate at N>=256); the attention
O-matmul (N=65) runs in bf16.
"""

import os
import sys
import types

import numpy as np

for _p in ("/opt/trn_rl_repo",):
    if _p not in sys.path and os.path.isdir(_p):
        sys.path.append(_p)

import concourse.bass as bass  # noqa: E402
import concourse.tile as tile  # noqa: E402
from concourse import bacc, mybir  # noqa: E402
from concourse.bass import ts  # noqa: E402
from concourse.bass_utils import run_bass_kernel_spmd  # noqa: E402
from concourse.masks import make_identity  # noqa: E402

# ---- problem constants (hardcoded per contract) ----
B_SZ, S, D = 8, 512, 768
H = 12
N_EXP = 8
RANK = 16
ALPHA = 32
TOP_K = 2
HD = D // H            # 64
T = S                  # tokens per core
NR = N_EXP * RANK      # 128
O3 = 3 * D             # 2304
N_CORES = 8

F32 = mybir.dt.float32
F32R = mybir.dt.float32r
BF16 = mybir.dt.bfloat16

DC = D // 128          # 6 d-chunks
TC = T // 128          # 4 token-chunks
QKC = (2 * D) // 128   # 12 o-chunks for q,k


def build_nc():
    """Build the SPMD program (same on all 8 cores)."""
    nc = bacc.Bacc("TRN2", target_bir_lowering=False, debug=False,
                   num_devices=N_CORES)

    dp = nc.declare_dram_parameter
    xT_d = dp("xT", [D, T], F32R, isOutput=False).ap()
    wqk_d = dp("wqkT", [QKC, 128, DC * 128], F32R, isOutput=False).ap()  # o-chunk-tiled, q pre-scaled
    wv_d = dp("wvT", [D, D], F32R, isOutput=False).ap()
    wg_d = dp("wgT", [D, N_EXP], F32R, isOutput=False).ap()
    at_d = dp("aT", [D, NR], F32R, isOutput=False).ap()         # pre-scaled by alpha/r
    btqk_d = dp("btqk", [NR, 2 * D], F32R, isOutput=False).ap()  # q pre-scaled
    bv_d = dp("bv", [NR, D], F32R, isOutput=False).ap()
    wp_d = dp("wpT", [D, D], F32R, isOutput=False).ap()
    bqk_d = dp("bqk", [128, QKC], F32, isOutput=False).ap()    # col o = b_qkv chunk
    bvv_d = dp("bvv", [1, D], F32, isOutput=False).ap()
    bg_d = dp("bg", [1, N_EXP], F32, isOutput=False).ap()
    bp_d = dp("bp", [1, D], F32, isOutput=False).ap()
    e8_d = dp("e8", [N_EXP, NR], F32R, isOutput=False).ap()     # expert->slot expand
    out_d = dp("out", [T, D], F32, isOutput=True).ap()

    with tile.TileContext(nc) as tc:
        _body(nc, tc, xT_d, wqk_d, wv_d, wg_d, at_d, btqk_d, bv_d, wp_d,
              bqk_d, bvv_d, bg_d, bp_d, e8_d, out_d)
    nc.compile()
    return nc


def _body(nc, tc, xT_d, wqk_d, wv_d, wg_d, at_d, btqk_d, bv_d, wp_d,
          bqk_d, bvv_d, bg_d, bp_d, e8_d, out_d):
    from contextlib import ExitStack
    ctx = ExitStack()
    with ctx:
        singles = ctx.enter_context(tc.tile_pool(name="singles", bufs=1))
        acts = ctx.enter_context(tc.tile_pool(name="acts", bufs=1))
        stx = ctx.enter_context(tc.tile_pool(name="stx", bufs=8))
        small_sb = ctx.enter_context(tc.tile_pool(name="small_sb", bufs=4))
        ps512 = ctx.enter_context(tc.tile_pool(name="ps512", bufs=8, space="PSUM"))

        # ---- load everything ----
        def load(pool, shape, src, name, dt=F32):
            t = pool.tile(shape, dt, tag=name, name=name)
            nc.sync.dma_start(out=t, in_=src)
            return t

        # order matters: the DMA queues drain in submit order, so put the
        # small tensors the router/lora prologue needs ahead of the big
        # weight matrices.
        xT = [load(singles, [128, T], xT_d[ts(k, 128), :], f"xT{k}", F32R) for k in range(DC)]
        wg = [load(singles, [128, N_EXP], wg_d[ts(k, 128), :], f"wg{k}", F32R) for k in range(DC)]
        aT = [load(singles, [128, NR], at_d[ts(k, 128), :], f"aT{k}", F32R) for k in range(DC)]
        e8 = load(singles, [N_EXP, NR], e8_d, "e8", F32R)
        bqk = load(singles, [128, QKC], bqk_d, "bqk")
        wqk = [None] * QKC
        for oc in (0, 6):
            wqk[oc] = load(singles, [128, DC * 128], wqk_d[oc], f"wqk{oc}", F32R)
        btqk = load(singles, [NR, 2 * D], btqk_d, "btqk", F32R)
        bv = load(singles, [NR, D], bv_d, "bv", F32R)
        for j in range(1, 6):
            for oc in (j, 6 + j):
                wqk[oc] = load(singles, [128, DC * 128], wqk_d[oc], f"wqk{oc}", F32R)
        wv = [load(singles, [128, D], wv_d[ts(k, 128), :], f"wv{k}", F32R) for k in range(DC)]
        wp = [load(singles, [128, D], wp_d[ts(k, 128), :], f"wp{k}", F32R) for k in range(DC)]

        # biases broadcast to 128 partitions via DMA (DRE replicate) so no
        # K=1 matmuls are needed
        bg_full = singles.tile([128, N_EXP], F32, tag="bg_full")
        nc.gpsimd.dma_start(out=bg_full, in_=bg_d.partition_broadcast(128)[:, 0, :])
        bvv_full = singles.tile([128, D], F32, tag="bvv_full")
        nc.gpsimd.dma_start(out=bvv_full, in_=bvv_d.partition_broadcast(128)[:, 0, :])
        bp_full = singles.tile([128, D], F32, tag="bp_full")
        nc.gpsimd.dma_start(out=bp_full, in_=bp_d.partition_broadcast(128)[:, 0, :])

        ident = singles.tile([128, 128], F32, tag="ident")
        make_identity(nc, ident)

        # ---- router: probs -> top2 dispatch [t, 8], then expand to [128 nr, t]
        #      all 4 token-chunks processed in ONE batched [128, 4, 8] DVE
        #      chain to keep the cT critical path short ----
        dispT = acts.tile([N_EXP, T], F32R, tag="dispT")
        lg = ps512.tile([128, TC, N_EXP], F32, tag="ps512")
        for tci in range(TC):
            for k in range(DC):
                nc.tensor.matmul(lg[:, tci, :], (xT[k][:, ts(tci, 128)]),
                                 (wg[k]), start=(k == 0), stop=(k == DC - 1))
        probs = small_sb.tile([128, TC, N_EXP], F32, tag="probs")
        nc.vector.tensor_add(probs, lg,
                             bg_full.unsqueeze(1).to_broadcast((128, TC, N_EXP)))
        nc.scalar.activation(probs, probs, mybir.ActivationFunctionType.Exp)
        sums = small_sb.tile([128, TC, 1], F32, tag="sums")
        nc.vector.reduce_sum(sums, probs, axis=mybir.AxisListType.X)
        recip = small_sb.tile([128, TC, 1], F32, tag="recip")
        nc.vector.reciprocal(recip, sums)
        nc.vector.tensor_mul(probs, probs, recip.to_broadcast((128, TC, N_EXP)))
        m1 = small_sb.tile([128, TC, 1], F32, tag="m1")
        nc.vector.reduce_max(m1, probs, axis=mybir.AxisListType.X)
        masked = small_sb.tile([128, TC, N_EXP], F32, tag="masked")
        nc.vector.tensor_tensor(masked, probs, m1.to_broadcast((128, TC, N_EXP)),
                                op=mybir.AluOpType.is_equal)
        nc.vector.tensor_scalar_mul(masked, masked, -10.0)
        nc.vector.tensor_add(masked, masked, probs)
        m2 = small_sb.tile([128, TC, 1], F32, tag="m2")
        nc.vector.reduce_max(m2, masked, axis=mybir.AxisListType.X)
        disp = small_sb.tile([128, TC, N_EXP], F32, tag="disp")
        nc.vector.tensor_tensor(disp, probs, m2.to_broadcast((128, TC, N_EXP)),
                                op=mybir.AluOpType.is_ge)
        nc.vector.tensor_mul(disp, disp, probs)
        # transpose each [128, 8] chunk -> [8, 128] into dispT
        for tci in range(TC):
            trp = ps512.tile([N_EXP, 128], F32, tag="ps512", name="trp")
            nc.tensor.transpose(trp, disp[:, tci, :], ident)
            nc.vector.tensor_copy(dispT[:, ts(tci, 128)], trp)

        # a_downT[nr, t] = (scaled A_flat) @ x  ;  cT = a_downT * expand(dispT)
        adn = ps512.tile([128, T], F32, tag="ps512")
        for k in range(DC):
            nc.tensor.matmul(adn, (aT[k]), (xT[k]),
                             start=(k == 0), stop=(k == DC - 1))
        adn_sb = acts.tile([128, T], F32, tag="adn_sb")
        nc.vector.tensor_copy(adn_sb, adn)
        expd = ps512.tile([128, T], F32, tag="ps512")
        nc.tensor.matmul(expd, (e8), (dispT), start=True, stop=True)
        cT = acts.tile([128, T], F32R, tag="cT")
        nc.vector.tensor_mul(cT, adn_sb, expd)

        # ---- v natural + ones column: v_aug[tc][128, 12*65] bf16 ----
        # (emitted before qkT so DMA order can put wv ahead of wqk if needed;
        #  scheduler reorders freely anyway)
        v_aug = []
        for tci in range(TC):
            pva = ps512.tile([128, 512], F32, tag="ps512", name="pva")
            pvb = ps512.tile([128, 256], F32, tag="ps512", name="pvb")
            for (pv, n0, nsz) in ((pva, 0, 512), (pvb, 512, 256)):
                for k in range(DC):
                    nc.tensor.matmul(pv[:, 0:nsz],
                                     (xT[k][:, ts(tci, 128)]),
                                     (wv[k][:, n0:n0 + nsz]),
                                     start=(k == 0), stop=False)
                nc.tensor.matmul(pv[:, 0:nsz], (cT[:, ts(tci, 128)]),
                                 (bv[:, n0:n0 + nsz]), start=False, stop=True)
            va = acts.tile([128, H, 2 * HD], BF16, tag=f"v_aug{tci}")
            nc.vector.tensor_add(va[:, 0:8, 0:HD],
                                 pva.rearrange("p (h c) -> p h c", c=HD),
                                 bvv_full[:, 0:512].rearrange(
                                     "p (h c) -> p h c", c=HD))
            nc.vector.tensor_add(va[:, 8:12, 0:HD],
                                 pvb.rearrange("p (h c) -> p h c", c=HD),
                                 bvv_full[:, 512:768].rearrange(
                                     "p (h c) -> p h c", c=HD))
            # 64 ones-columns: the O-matmul then lands Z on psum partitions
            # 64:128, so the softmax normalizer needs no cross-partition move
            nc.vector.memset(va[:, :, HD:2 * HD], 1.0)
            v_aug.append(va)

        # ---- q,k transposed (pair-ordered: q chunk j then k chunk j so the
        #      attention for head pair j can start as early as possible) ----
        qkT = [None] * QKC
        for j in range(QKC // 2):
            for oc in (j, 6 + j):
                pq = ps512.tile([128, T], F32, tag="ps512", name="pq")
                for k in range(DC):
                    nc.tensor.matmul(pq, (wqk[oc][:, ts(k, 128)]), (xT[k]),
                                     start=(k == 0), stop=False)
                nc.tensor.matmul(pq, (btqk[:, ts(oc, 128)]), (cT),
                                 start=False, stop=True)
                sb = acts.tile([128, T], BF16, tag=f"qkT{oc}", name=f"qkT{oc}")
                nc.vector.tensor_scalar_add(sb, pq, bqk[:, oc:oc + 1])
                qkT[oc] = sb

        # ---- attention (per head: scores transposed, exp, then
        #      attn_outT = v_aug.T @ st_exp directly in [d, t] layout with the
        #      softmax normalizer arriving as the ones-column row) ----
        aoT = [acts.tile([128, T], F32R, tag=f"aoT{dc}", name=f"aoT{dc}")
               for dc in range(DC)]
        for j in range(H // 2):
            # heads 2j (rows 0:64) and 2j+1 (rows 64:128) of qkT tile pair j:
            # emit their K=64 score matmuls back-to-back so the row-disjoint
            # pair packs onto the PE array concurrently.
            qt = qkT[j]
            kt = qkT[6 + j]
            st_exp = {0: [], 64: []}
            for kc in range(TC):
                for po in (0, 64):
                    pst = ps512.tile([128, T], F32, tag="ps512", name="pst")
                    nc.tensor.matmul(pst, (kt[po:po + HD, ts(kc, 128)]),
                                     (qt[po:po + HD, :]), start=True, stop=True,
                                     tile_position=(po, 0))
                    se = stx.tile([128, T], BF16, tag="st_exp", name="se")
                    nc.scalar.activation(se, pst,
                                         mybir.ActivationFunctionType.Exp)
                    st_exp[po].append(se)
            for po in (0, 64):
                h = 2 * j + po // 64
                pot = ps512.tile([128, T], F32, tag="ps512", name="pot")
                for kc in range(TC):
                    nc.tensor.matmul(pot, v_aug[kc][:, h, :], st_exp[po][kc],
                                     start=(kc == 0), stop=(kc == TC - 1))
                # rows 64:128 of pot are 64 copies of Z[q]
                rzb = small_sb.tile([HD, T], F32, tag="rzb", name="rzb")
                nc.vector.reciprocal(rzb, pot[HD:2 * HD, :])
                nc.vector.tensor_mul(aoT[j][po:po + HD, :], pot[0:HD, :], rzb)

        # ---- final projection ----
        for tci in range(TC):
            pfa = ps512.tile([128, 512], F32, tag="ps512", name="pfa")
            pfb = ps512.tile([128, 256], F32, tag="ps512", name="pfb")
            for (pf, n0, nsz) in ((pfa, 0, 512), (pfb, 512, 256)):
                for dc in range(DC):
                    nc.tensor.matmul(pf[:, 0:nsz],
                                     (aoT[dc][:, ts(tci, 128)]),
                                     (wp[dc][:, n0:n0 + nsz]),
                                     start=(dc == 0), stop=(dc == DC - 1))
            osb = acts.tile([128, D], F32, tag=f"out_sb{tci}")
            nc.vector.tensor_add(osb[:, 0:512], pfa, bp_full[:, 0:512])
            nc.vector.tensor_add(osb[:, 512:768], pfb, bp_full[:, 512:768])
            nc.sync.dma_start(out=out_d[ts(tci, 128), :], in_=osb)


def prep_inputs(x, W_qkv, b_qkv, W_gate, b_gate, A, B_lora, W_proj, b_proj):
    """Host-side prep: pre-transpose/pre-scale weights, shard x by batch."""
    scale = HD ** -0.5
    scaling = ALPHA / RANK
    W_qkv = np.asarray(W_qkv, np.float32).copy()
    b_qkv = np.asarray(b_qkv, np.float32).copy()
    B_lora = np.asarray(B_lora, np.float32).copy()
    W_qkv[:D] *= scale          # fold attention scale into q
    b_qkv[:D] *= scale
    B_lora[:, :D, :] *= scale

    wqkT = W_qkv[:2 * D].T                                      # [768, 1536]
    # o-chunk-tiled: wqk_tiled[oc, p, k*128+f] = wqkT[k*128+p, oc*128+f]
    wqk_tiled = np.ascontiguousarray(
        wqkT.reshape(DC, 128, QKC, 128).transpose(2, 1, 0, 3).reshape(
            QKC, 128, DC * 128))
    wvT = np.ascontiguousarray(W_qkv[2 * D:].T)                 # [768, 768]
    wgT = np.ascontiguousarray(np.asarray(W_gate, np.float32).T)  # [768, 8]
    aT = np.ascontiguousarray(
        (np.asarray(A, np.float32).reshape(NR, D) * scaling).T)  # [768, 128]
    bt = np.ascontiguousarray(
        B_lora.transpose(0, 2, 1).reshape(NR, O3))               # [128, 2304]
    btqk = np.ascontiguousarray(bt[:, :2 * D])
    bvm = np.ascontiguousarray(bt[:, 2 * D:])
    wpT = np.ascontiguousarray(np.asarray(W_proj, np.float32).T)
    bqk = np.ascontiguousarray(b_qkv[:2 * D].reshape(QKC, 128).T)  # [128, 12]
    bvv = np.ascontiguousarray(b_qkv[2 * D:].reshape(1, D))
    bg = np.ascontiguousarray(np.asarray(b_gate, np.float32).reshape(1, N_EXP))
    bp = np.ascontiguousarray(np.asarray(b_proj, np.float32).reshape(1, D))
    e8 = np.ascontiguousarray(np.repeat(np.eye(N_EXP, dtype=np.float32), RANK, axis=1))

    shared = dict(wqkT=wqk_tiled, wvT=wvT, wgT=wgT, aT=aT, btqk=btqk, bv=bvm,
                  wpT=wpT, bqk=bqk, bvv=bvv, bg=bg, bp=bp, e8=e8)
    x = np.asarray(x, np.float32)
    in_maps = []
    for b in range(N_CORES):
        m = dict(shared)
        m["xT"] = np.ascontiguousarray(x[b].T)
        in_maps.append(m)
    return in_maps


def _install_ntff_shim():
    """run_bass_kernel_spmd(trace=True) under axon needs antenv.axon_hooks."""
    if "antenv.axon_hooks" in sys.modules:
        return
    try:
        from trn_agent_boot.trn_boot import _ntff_profile_via_ctypes
        hook = _ntff_profile_via_ctypes("/opt/axon/libaxon_pjrt.so")
    except Exception:
        hook = None
    mod = types.ModuleType("antenv.axon_hooks")
    mod.get_axon_ntff_profile_hook = lambda: hook
    mod.set_axon_ntff_profile_hook = lambda h: None
    sys.modules["antenv.axon_hooks"] = mod


_NC_CACHE = None


def kernel(x, W_qkv, b_qkv, W_gate, b_gate, A, B_lora, W_proj, b_proj,
           _trace=False):
    global _NC_CACHE
    if _NC_CACHE is None:
        _NC_CACHE = build_nc()
    nc = _NC_CACHE
    in_maps = prep_inputs(x, W_qkv, b_qkv, W_gate, b_gate, A, B_lora,
                          W_proj, b_proj)
    if _trace:
        _install_ntff_shim()
    res = run_bass_kernel_spmd(nc, in_maps, list(range(N_CORES)), trace=_trace)
    out = np.stack([res.results[i]["out"] for i in range(N_CORES)], axis=0)
    out = out.reshape(B_SZ, S, D)
    if _trace:
        kernel.last_exec_time_ns = res.exec_time_ns
        kernel.last_results = res
    return out


# revision 15
# speedup vs baseline: 1.2611x; 1.2611x over previous
"""MoE-LoRA fused attention kernel for 8 Trainium2 NeuronCores.

Problem: x[8,512,768] -> qkv = x@W_qkv.T + top2-routed LoRA experts;
multi-head attention (12 heads, hd=64); out-projection.

Sharding: data-parallel over batch. Core b handles batch element b
(attention + routing are token-local, so there is no cross-core
communication at all).

Per-core layout strategy (everything host-pre-transposed so the device
does no transposes on the forward path):
  xT      [768, 512]   (d on partitions, t free)
  q,k     computed transposed:  qkT[o, t] = sum_d W[o,d] xT[d,t]
  v       computed natural:     v[t, o]
  scores  computed transposed:  st[kt, q] = kT.T @ qT  (exp is elementwise;
          the softmax normalizer Z[q] = sum_k exp(st) falls out of the
          O-matmul as a ones-column appended to v)
  O       computed natural:     O[q, hd|Z] = st_exp.T @ [v | 1]
  proj    needs attn_out transposed -> 24 PE transposes, then
          final[t, o] = attn_outT.T @ W_projT
Matmuls run as float32r (full PE rate at N>=256); the attention
O-matmul (N=65) runs in bf16.
"""

import os
import sys
import types

import numpy as np

for _p in ("/opt/trn_rl_repo",):
    if _p not in sys.path and os.path.isdir(_p):
        sys.path.append(_p)

import concourse.bass as bass  # noqa: E402
import concourse.tile as tile  # noqa: E402
from concourse import bacc, mybir  # noqa: E402
from concourse.bass import ts  # noqa: E402
from concourse.bass_utils import run_bass_kernel_spmd  # noqa: E402
from concourse.masks import make_identity  # noqa: E402

# ---- problem constants (hardcoded per contract) ----
B_SZ, S, D = 8, 512, 768
H = 12
N_EXP = 8
RANK = 16
ALPHA = 32
TOP_K = 2
HD = D // H            # 64
T = S                  # tokens per core
NR = N_EXP * RANK      # 128
O3 = 3 * D             # 2304
N_CORES = 8

F32 = mybir.dt.float32
F32R = mybir.dt.float32r
BF16 = mybir.dt.bfloat16

DC = D // 128          # 6 d-chunks
TC = T // 128          # 4 token-chunks
QKC = (2 * D) // 128   # 12 o-chunks for q,k


def build_nc():
    """Build the SPMD program (same on all 8 cores)."""
    nc = bacc.Bacc("TRN2", target_bir_lowering=False, debug=False,
                   num_devices=N_CORES)

    dp = nc.declare_dram_parameter
    xT_d = dp("xT", [D, T], F32R, isOutput=False).ap()
    wqk_d = dp("wqkT", [QKC, 128, DC * 128], F32R, isOutput=False).ap()  # o-chunk-tiled, q pre-scaled
    wv_d = dp("wvT", [D, D], F32R, isOutput=False).ap()
    wg_d = dp("wgT", [D, N_EXP], F32R, isOutput=False).ap()
    at_d = dp("aT", [D, NR], F32R, isOutput=False).ap()         # pre-scaled by alpha/r
    btqk_d = dp("btqk", [NR, 2 * D], F32R, isOutput=False).ap()  # q pre-scaled
    bv_d = dp("bv", [NR, D], F32R, isOutput=False).ap()
    wp_d = dp("wpT", [D, D], F32R, isOutput=False).ap()
    bqk_d = dp("bqk", [128, QKC], F32, isOutput=False).ap()    # col o = b_qkv chunk
    bvv_d = dp("bvv", [1, D], F32, isOutput=False).ap()
    bg_d = dp("bg", [1, N_EXP], F32, isOutput=False).ap()
    bp_d = dp("bp", [1, D], F32, isOutput=False).ap()
    e8_d = dp("e8", [N_EXP, NR], F32R, isOutput=False).ap()     # expert->slot expand
    out_d = dp("out", [T, D], F32, isOutput=True).ap()

    with tile.TileContext(nc) as tc:
        _body(nc, tc, xT_d, wqk_d, wv_d, wg_d, at_d, btqk_d, bv_d, wp_d,
              bqk_d, bvv_d, bg_d, bp_d, e8_d, out_d)
    nc.compile()
    return nc


def _body(nc, tc, xT_d, wqk_d, wv_d, wg_d, at_d, btqk_d, bv_d, wp_d,
          bqk_d, bvv_d, bg_d, bp_d, e8_d, out_d):
    from contextlib import ExitStack
    ctx = ExitStack()
    with ctx:
        singles = ctx.enter_context(tc.tile_pool(name="singles", bufs=1))
        acts = ctx.enter_context(tc.tile_pool(name="acts", bufs=1))
        stx = ctx.enter_context(tc.tile_pool(name="stx", bufs=16))
        small_sb = ctx.enter_context(tc.tile_pool(name="small_sb", bufs=4))
        ps512 = ctx.enter_context(tc.tile_pool(name="ps512", bufs=8, space="PSUM"))

        # ---- load everything ----
        def load(pool, shape, src, name, dt=F32):
            t = pool.tile(shape, dt, tag=name, name=name)
            nc.sync.dma_start(out=t, in_=src)
            return t

        # order matters: the DMA queues drain in submit order, so put the
        # small tensors the router/lora prologue needs ahead of the big
        # weight matrices.
        xT = [load(singles, [128, T], xT_d[ts(k, 128), :], f"xT{k}", F32R) for k in range(DC)]
        wg = [load(singles, [128, N_EXP], wg_d[ts(k, 128), :], f"wg{k}", F32R) for k in range(DC)]
        aT = [load(singles, [128, NR], at_d[ts(k, 128), :], f"aT{k}", F32R) for k in range(DC)]
        e8 = load(singles, [N_EXP, NR], e8_d, "e8", F32R)
        bqk = load(singles, [128, QKC], bqk_d, "bqk")
        bv = load(singles, [NR, D], bv_d, "bv", F32R)
        wv = [load(singles, [128, D], wv_d[ts(k, 128), :], f"wv{k}", F32R) for k in range(DC)]
        wqk = [None] * QKC
        for oc in (0, 6):
            wqk[oc] = load(singles, [128, DC * 128], wqk_d[oc], f"wqk{oc}", F32R)
        btqk = load(singles, [NR, 2 * D], btqk_d, "btqk", F32R)
        for j in range(1, 6):
            for oc in (j, 6 + j):
                wqk[oc] = load(singles, [128, DC * 128], wqk_d[oc], f"wqk{oc}", F32R)
        wp = [load(singles, [128, D], wp_d[ts(k, 128), :], f"wp{k}", F32R) for k in range(DC)]

        # biases broadcast to 128 partitions via DMA (DRE replicate) so no
        # K=1 matmuls are needed
        bg_full = singles.tile([128, N_EXP], F32, tag="bg_full")
        nc.gpsimd.dma_start(out=bg_full, in_=bg_d.partition_broadcast(128)[:, 0, :])
        bvv_full = singles.tile([128, D], F32, tag="bvv_full")
        nc.gpsimd.dma_start(out=bvv_full, in_=bvv_d.partition_broadcast(128)[:, 0, :])
        bp_full = singles.tile([128, D], F32, tag="bp_full")
        nc.gpsimd.dma_start(out=bp_full, in_=bp_d.partition_broadcast(128)[:, 0, :])

        ident = singles.tile([128, 128], F32, tag="ident")
        make_identity(nc, ident)

        # ---- router: probs -> top2 dispatch [t, 8], then expand to [128 nr, t]
        #      all 4 token-chunks processed in ONE batched [128, 4, 8] DVE
        #      chain to keep the cT critical path short ----
        dispT = acts.tile([N_EXP, T], F32R, tag="dispT")
        lg = ps512.tile([128, TC, N_EXP], F32, tag="ps512")
        for tci in range(TC):
            for k in range(DC):
                nc.tensor.matmul(lg[:, tci, :], (xT[k][:, ts(tci, 128)]),
                                 (wg[k]), start=(k == 0), stop=(k == DC - 1))
        probs = small_sb.tile([128, TC, N_EXP], F32, tag="probs")
        nc.vector.tensor_add(probs, lg,
                             bg_full.unsqueeze(1).to_broadcast((128, TC, N_EXP)))
        nc.scalar.activation(probs, probs, mybir.ActivationFunctionType.Exp)
        sums = small_sb.tile([128, TC, 1], F32, tag="sums")
        nc.vector.reduce_sum(sums, probs, axis=mybir.AxisListType.X)
        recip = small_sb.tile([128, TC, 1], F32, tag="recip")
        nc.vector.reciprocal(recip, sums)
        nc.vector.tensor_mul(probs, probs, recip.to_broadcast((128, TC, N_EXP)))
        m1 = small_sb.tile([128, TC, 1], F32, tag="m1")
        nc.vector.reduce_max(m1, probs, axis=mybir.AxisListType.X)
        masked = small_sb.tile([128, TC, N_EXP], F32, tag="masked")
        nc.vector.tensor_tensor(masked, probs, m1.to_broadcast((128, TC, N_EXP)),
                                op=mybir.AluOpType.is_equal)
        nc.vector.tensor_scalar_mul(masked, masked, -10.0)
        nc.vector.tensor_add(masked, masked, probs)
        m2 = small_sb.tile([128, TC, 1], F32, tag="m2")
        nc.vector.reduce_max(m2, masked, axis=mybir.AxisListType.X)
        disp = small_sb.tile([128, TC, N_EXP], F32, tag="disp")
        nc.vector.tensor_tensor(disp, probs, m2.to_broadcast((128, TC, N_EXP)),
                                op=mybir.AluOpType.is_ge)
        nc.vector.tensor_mul(disp, disp, probs)
        # transpose each [128, 8] chunk -> [8, 128] into dispT
        for tci in range(TC):
            trp = ps512.tile([N_EXP, 128], F32, tag="ps512", name="trp")
            nc.tensor.transpose(trp, disp[:, tci, :], ident)
            nc.vector.tensor_copy(dispT[:, ts(tci, 128)], trp)

        # a_downT[nr, t] = (scaled A_flat) @ x  ;  cT = a_downT * expand(dispT)
        adn = ps512.tile([128, T], F32, tag="ps512")
        for k in range(DC):
            nc.tensor.matmul(adn, (aT[k]), (xT[k]),
                             start=(k == 0), stop=(k == DC - 1))
        adn_sb = acts.tile([128, T], F32, tag="adn_sb")
        nc.vector.tensor_copy(adn_sb, adn)
        expd = ps512.tile([128, T], F32, tag="ps512")
        nc.tensor.matmul(expd, (e8), (dispT), start=True, stop=True)
        cT = acts.tile([128, T], F32R, tag="cT")
        nc.vector.tensor_mul(cT, adn_sb, expd)

        # ---- v natural + ones columns: v_aug[tc][128, 12, 128] bf16 ----
        v_aug = []
        for tci in range(TC):
            pva = ps512.tile([128, 512], F32, tag="ps512", name="pva")
            pvb = ps512.tile([128, 256], F32, tag="ps512", name="pvb")
            for (pv, n0, nsz) in ((pva, 0, 512), (pvb, 512, 256)):
                for k in range(DC):
                    nc.tensor.matmul(pv[:, 0:nsz],
                                     (xT[k][:, ts(tci, 128)]),
                                     (wv[k][:, n0:n0 + nsz]),
                                     start=(k == 0), stop=False)
                nc.tensor.matmul(pv[:, 0:nsz], (cT[:, ts(tci, 128)]),
                                 (bv[:, n0:n0 + nsz]), start=False, stop=True)
            va = acts.tile([128, H, 2 * HD], BF16, tag=f"v_aug{tci}")
            nc.vector.tensor_add(va[:, 0:8, 0:HD],
                                 pva.rearrange("p (h c) -> p h c", c=HD),
                                 bvv_full[:, 0:512].rearrange(
                                     "p (h c) -> p h c", c=HD))
            nc.vector.tensor_add(va[:, 8:12, 0:HD],
                                 pvb.rearrange("p (h c) -> p h c", c=HD),
                                 bvv_full[:, 512:768].rearrange(
                                     "p (h c) -> p h c", c=HD))
            # 64 ones-columns: the O-matmul then lands Z on psum partitions
            # 64:128, so the softmax normalizer needs no cross-partition move
            nc.vector.memset(va[:, :, HD:2 * HD], 1.0)
            v_aug.append(va)

        # ---- fused qk-projection + attention, software-pipelined by head
        #      pair: emit pair j's qkT matmuls and score/exp stage, then pair
        #      j-1's output matmuls, so the PE never waits on the ACT exps ----
        qkT = [None] * QKC
        aoT = [acts.tile([128, T], F32R, tag=f"aoT{dc}", name=f"aoT{dc}")
               for dc in range(DC)]

        def emit_qk_pair(j):
            for oc in (j, 6 + j):
                pq = ps512.tile([128, T], F32, tag="ps512", name="pq")
                for k in range(DC):
                    nc.tensor.matmul(pq, (wqk[oc][:, ts(k, 128)]), (xT[k]),
                                     start=(k == 0), stop=False)
                nc.tensor.matmul(pq, (btqk[:, ts(oc, 128)]), (cT),
                                 start=False, stop=True)
                sb = acts.tile([128, T], BF16, tag=f"qkT{oc}", name=f"qkT{oc}")
                nc.vector.tensor_scalar_add(sb, pq, bqk[:, oc:oc + 1])
                qkT[oc] = sb

        def emit_st(j):
            qt = qkT[j]
            kt = qkT[6 + j]
            st_exp = {0: [], 64: []}
            for kc in range(TC):
                for po in (0, 64):
                    pst = ps512.tile([128, T], F32, tag="ps512", name="pst")
                    nc.tensor.matmul(pst, (kt[po:po + HD, ts(kc, 128)]),
                                     (qt[po:po + HD, :]), start=True, stop=True,
                                     tile_position=(po, 0))
                    se = stx.tile([128, T], BF16, tag="st_exp", name="se")
                    nc.scalar.activation(se, pst,
                                         mybir.ActivationFunctionType.Exp)
                    st_exp[po].append(se)
            return st_exp

        def emit_ot(j, st_exp):
            for po in (0, 64):
                h = 2 * j + po // 64
                pot = ps512.tile([128, T], F32, tag="ps512", name="pot")
                for kc in range(TC):
                    nc.tensor.matmul(pot, v_aug[kc][:, h, :], st_exp[po][kc],
                                     start=(kc == 0), stop=(kc == TC - 1))
                # rows 64:128 of pot are 64 copies of Z[q]
                rzb = small_sb.tile([HD, T], F32, tag="rzb", name="rzb")
                nc.vector.reciprocal(rzb, pot[HD:2 * HD, :])
                nc.vector.tensor_mul(aoT[j][po:po + HD, :], pot[0:HD, :], rzb)

        emit_qk_pair(0)
        prev = (0, emit_st(0))
        for j in range(1, H // 2):
            emit_qk_pair(j)
            cur = (j, emit_st(j))
            emit_ot(*prev)
            prev = cur
        emit_ot(*prev)

        # ---- final projection ----
        for tci in range(TC):
            pfa = ps512.tile([128, 512], F32, tag="ps512", name="pfa")
            pfb = ps512.tile([128, 256], F32, tag="ps512", name="pfb")
            for (pf, n0, nsz) in ((pfa, 0, 512), (pfb, 512, 256)):
                for dc in range(DC):
                    nc.tensor.matmul(pf[:, 0:nsz],
                                     (aoT[dc][:, ts(tci, 128)]),
                                     (wp[dc][:, n0:n0 + nsz]),
                                     start=(dc == 0), stop=(dc == DC - 1))
            osb = acts.tile([128, D], F32, tag=f"out_sb{tci}")
            nc.vector.tensor_add(osb[:, 0:512], pfa, bp_full[:, 0:512])
            nc.vector.tensor_add(osb[:, 512:768], pfb, bp_full[:, 512:768])
            nc.sync.dma_start(out=out_d[ts(tci, 128), :], in_=osb)


def prep_inputs(x, W_qkv, b_qkv, W_gate, b_gate, A, B_lora, W_proj, b_proj):
    """Host-side prep: pre-transpose/pre-scale weights, shard x by batch."""
    scale = HD ** -0.5
    scaling = ALPHA / RANK
    W_qkv = np.asarray(W_qkv, np.float32).copy()
    b_qkv = np.asarray(b_qkv, np.float32).copy()
    B_lora = np.asarray(B_lora, np.float32).copy()
    W_qkv[:D] *= scale          # fold attention scale into q
    b_qkv[:D] *= scale
    B_lora[:, :D, :] *= scale

    wqkT = W_qkv[:2 * D].T                                      # [768, 1536]
    # o-chunk-tiled: wqk_tiled[oc, p, k*128+f] = wqkT[k*128+p, oc*128+f]
    wqk_tiled = np.ascontiguousarray(
        wqkT.reshape(DC, 128, QKC, 128).transpose(2, 1, 0, 3).reshape(
            QKC, 128, DC * 128))
    wvT = np.ascontiguousarray(W_qkv[2 * D:].T)                 # [768, 768]
    wgT = np.ascontiguousarray(np.asarray(W_gate, np.float32).T)  # [768, 8]
    aT = np.ascontiguousarray(
        (np.asarray(A, np.float32).reshape(NR, D) * scaling).T)  # [768, 128]
    bt = np.ascontiguousarray(
        B_lora.transpose(0, 2, 1).reshape(NR, O3))               # [128, 2304]
    btqk = np.ascontiguousarray(bt[:, :2 * D])
    bvm = np.ascontiguousarray(bt[:, 2 * D:])
    wpT = np.ascontiguousarray(np.asarray(W_proj, np.float32).T)
    bqk = np.ascontiguousarray(b_qkv[:2 * D].reshape(QKC, 128).T)  # [128, 12]
    bvv = np.ascontiguousarray(b_qkv[2 * D:].reshape(1, D))
    bg = np.ascontiguousarray(np.asarray(b_gate, np.float32).reshape(1, N_EXP))
    bp = np.ascontiguousarray(np.asarray(b_proj, np.float32).reshape(1, D))
    e8 = np.ascontiguousarray(np.repeat(np.eye(N_EXP, dtype=np.float32), RANK, axis=1))

    shared = dict(wqkT=wqk_tiled, wvT=wvT, wgT=wgT, aT=aT, btqk=btqk, bv=bvm,
                  wpT=wpT, bqk=bqk, bvv=bvv, bg=bg, bp=bp, e8=e8)
    x = np.asarray(x, np.float32)
    in_maps = []
    for b in range(N_CORES):
        m = dict(shared)
        m["xT"] = np.ascontiguousarray(x[b].T)
        in_maps.append(m)
    return in_maps


def _install_ntff_shim():
    """run_bass_kernel_spmd(trace=True) under axon needs antenv.axon_hooks."""
    if "antenv.axon_hooks" in sys.modules:
        return
    try:
        from trn_agent_boot.trn_boot import _ntff_profile_via_ctypes
        hook = _ntff_profile_via_ctypes("/opt/axon/libaxon_pjrt.so")
    except Exception:
        hook = None
    mod = types.ModuleType("antenv.axon_hooks")
    mod.get_axon_ntff_profile_hook = lambda: hook
    mod.set_axon_ntff_profile_hook = lambda h: None
    sys.modules["antenv.axon_hooks"] = mod


_NC_CACHE = None


def kernel(x, W_qkv, b_qkv, W_gate, b_gate, A, B_lora, W_proj, b_proj,
           _trace=False):
    global _NC_CACHE
    if _NC_CACHE is None:
        _NC_CACHE = build_nc()
    nc = _NC_CACHE
    in_maps = prep_inputs(x, W_qkv, b_qkv, W_gate, b_gate, A, B_lora,
                          W_proj, b_proj)
    if _trace:
        _install_ntff_shim()
    res = run_bass_kernel_spmd(nc, in_maps, list(range(N_CORES)), trace=_trace)
    out = np.stack([res.results[i]["out"] for i in range(N_CORES)], axis=0)
    out = out.reshape(B_SZ, S, D)
    if _trace:
        kernel.last_exec_time_ns = res.exec_time_ns
        kernel.last_results = res
    return out


# revision 17
# speedup vs baseline: 1.3697x; 1.0861x over previous
"""MoE-LoRA fused attention kernel for 8 Trainium2 NeuronCores.

Problem: x[8,512,768] -> qkv = x@W_qkv.T + top2-routed LoRA experts;
multi-head attention (12 heads, hd=64); out-projection.

Sharding: data-parallel over batch. Core b handles batch element b
(attention + routing are token-local, so there is no cross-core
communication at all).

Per-core layout strategy (everything host-pre-transposed so the device
does no transposes on the forward path):
  xT      [768, 512]   (d on partitions, t free)
  q,k     computed transposed:  qkT[o, t] = sum_d W[o,d] xT[d,t]
  v       computed natural:     v[t, o]
  scores  computed transposed:  st[kt, q] = kT.T @ qT  (exp is elementwise;
          the softmax normalizer Z[q] = sum_k exp(st) falls out of the
          O-matmul as a ones-column appended to v)
  O       computed natural:     O[q, hd|Z] = st_exp.T @ [v | 1]
  proj    needs attn_out transposed -> 24 PE transposes, then
          final[t, o] = attn_outT.T @ W_projT
Matmuls run as float32r (full PE rate at N>=256); the attention
O-matmul (N=65) runs in bf16.
"""

import os
import sys
import types

import numpy as np

for _p in ("/opt/trn_rl_repo",):
    if _p not in sys.path and os.path.isdir(_p):
        sys.path.append(_p)

import concourse.bass as bass  # noqa: E402
import concourse.tile as tile  # noqa: E402
from concourse import bacc, mybir  # noqa: E402
from concourse.bass import ts  # noqa: E402
from concourse.bass_utils import run_bass_kernel_spmd  # noqa: E402
from concourse.masks import make_identity  # noqa: E402

# ---- problem constants (hardcoded per contract) ----
B_SZ, S, D = 8, 512, 768
H = 12
N_EXP = 8
RANK = 16
ALPHA = 32
TOP_K = 2
HD = D // H            # 64
T = S                  # tokens per core
NR = N_EXP * RANK      # 128
O3 = 3 * D             # 2304
N_CORES = 8

F32 = mybir.dt.float32
F32R = mybir.dt.float32r
BF16 = mybir.dt.bfloat16

DC = D // 128          # 6 d-chunks
TC = T // 128          # 4 token-chunks
QKC = (2 * D) // 128   # 12 o-chunks for q,k


def build_nc():
    """Build the SPMD program (same on all 8 cores)."""
    nc = bacc.Bacc("TRN2", target_bir_lowering=False, debug=False,
                   num_devices=N_CORES)

    dp = nc.declare_dram_parameter
    xT_d = dp("xT", [D, T], F32R, isOutput=False).ap()
    wqk_d = dp("wqkT", [QKC, 128, DC * 128], F32R, isOutput=False).ap()  # o-chunk-tiled, q pre-scaled
    wv_d = dp("wvT", [D, D], F32R, isOutput=False).ap()
    wg_d = dp("wgT", [D, N_EXP], F32R, isOutput=False).ap()
    at_d = dp("aT", [D, NR], F32R, isOutput=False).ap()         # pre-scaled by alpha/r
    btqk_d = dp("btqk", [NR, 2 * D], F32R, isOutput=False).ap()  # q pre-scaled
    bv_d = dp("bv", [NR, D], F32R, isOutput=False).ap()
    wp_d = dp("wpT", [D, D], F32R, isOutput=False).ap()
    bqk_d = dp("bqk", [128, QKC], F32, isOutput=False).ap()    # col o = b_qkv chunk
    bvv_d = dp("bvv", [1, D], F32, isOutput=False).ap()
    bg_d = dp("bg", [1, N_EXP], F32, isOutput=False).ap()
    bp_d = dp("bp", [1, D], F32, isOutput=False).ap()
    e8_d = dp("e8", [N_EXP, NR], F32R, isOutput=False).ap()     # expert->slot expand
    out_d = dp("out", [T, D], F32, isOutput=True).ap()

    with tile.TileContext(nc) as tc:
        _body(nc, tc, xT_d, wqk_d, wv_d, wg_d, at_d, btqk_d, bv_d, wp_d,
              bqk_d, bvv_d, bg_d, bp_d, e8_d, out_d)
    nc.compile()
    return nc


def _body(nc, tc, xT_d, wqk_d, wv_d, wg_d, at_d, btqk_d, bv_d, wp_d,
          bqk_d, bvv_d, bg_d, bp_d, e8_d, out_d):
    from contextlib import ExitStack
    ctx = ExitStack()
    with ctx:
        singles = ctx.enter_context(tc.tile_pool(name="singles", bufs=1))
        acts = ctx.enter_context(tc.tile_pool(name="acts", bufs=1))
        stx = ctx.enter_context(tc.tile_pool(name="stx", bufs=16))
        small_sb = ctx.enter_context(tc.tile_pool(name="small_sb", bufs=4))
        ps512 = ctx.enter_context(tc.tile_pool(name="ps512", bufs=8, space="PSUM"))

        # ---- load everything ----
        def load(pool, shape, src, name, dt=F32):
            t = pool.tile(shape, dt, tag=name, name=name)
            nc.sync.dma_start(out=t, in_=src)
            return t

        # order matters: the DMA queues drain in submit order, so put the
        # small tensors the router/lora prologue needs ahead of the big
        # weight matrices.
        xT = [load(singles, [128, T], xT_d[ts(k, 128), :], f"xT{k}", F32R) for k in range(DC)]
        wg = [load(singles, [128, N_EXP], wg_d[ts(k, 128), :], f"wg{k}", F32R) for k in range(DC)]
        aT = [load(singles, [128, NR], at_d[ts(k, 128), :], f"aT{k}", F32R) for k in range(DC)]
        e8 = load(singles, [N_EXP, NR], e8_d, "e8", F32R)
        bqk = load(singles, [128, QKC], bqk_d, "bqk")
        bv = load(singles, [NR, D], bv_d, "bv", F32R)
        wv = [load(singles, [128, D], wv_d[ts(k, 128), :], f"wv{k}", F32R) for k in range(DC)]
        wqk = [None] * QKC
        for oc in (0, 6):
            wqk[oc] = load(singles, [128, DC * 128], wqk_d[oc], f"wqk{oc}", F32R)
        btqk = load(singles, [NR, 2 * D], btqk_d, "btqk", F32R)
        for j in range(1, 6):
            for oc in (j, 6 + j):
                wqk[oc] = load(singles, [128, DC * 128], wqk_d[oc], f"wqk{oc}", F32R)
        wp = [load(singles, [128, D], wp_d[ts(k, 128), :], f"wp{k}", F32R) for k in range(DC)]

        # biases broadcast to 128 partitions via DMA (DRE replicate) so no
        # K=1 matmuls are needed
        bg_full = singles.tile([128, N_EXP], F32, tag="bg_full")
        nc.gpsimd.dma_start(out=bg_full, in_=bg_d.partition_broadcast(128)[:, 0, :])
        bvv_full = singles.tile([128, D], F32, tag="bvv_full")
        nc.gpsimd.dma_start(out=bvv_full, in_=bvv_d.partition_broadcast(128)[:, 0, :])
        bp_full = singles.tile([128, D], F32, tag="bp_full")
        nc.gpsimd.dma_start(out=bp_full, in_=bp_d.partition_broadcast(128)[:, 0, :])

        ident = singles.tile([128, 128], F32, tag="ident")
        make_identity(nc, ident)

        # ---- router: probs -> top2 dispatch [t, 8], then expand to [128 nr, t]
        #      all 4 token-chunks processed in ONE batched [128, 4, 8] DVE
        #      chain to keep the cT critical path short ----
        dispT = acts.tile([N_EXP, T], F32R, tag="dispT")
        lg = ps512.tile([128, TC, N_EXP], F32, tag="ps512")
        for tci in range(TC):
            for k in range(DC):
                nc.tensor.matmul(lg[:, tci, :], (xT[k][:, ts(tci, 128)]),
                                 (wg[k]), start=(k == 0), stop=(k == DC - 1))
        probs = small_sb.tile([128, TC, N_EXP], F32, tag="probs")
        nc.vector.tensor_add(probs, lg,
                             bg_full.unsqueeze(1).to_broadcast((128, TC, N_EXP)))
        nc.scalar.activation(probs, probs, mybir.ActivationFunctionType.Exp)
        sums = small_sb.tile([128, TC, 1], F32, tag="sums")
        nc.vector.reduce_sum(sums, probs, axis=mybir.AxisListType.X)
        recip = small_sb.tile([128, TC, 1], F32, tag="recip")
        nc.vector.reciprocal(recip, sums)
        nc.vector.tensor_mul(probs, probs, recip.to_broadcast((128, TC, N_EXP)))
        m1 = small_sb.tile([128, TC, 1], F32, tag="m1")
        nc.vector.reduce_max(m1, probs, axis=mybir.AxisListType.X)
        masked = small_sb.tile([128, TC, N_EXP], F32, tag="masked")
        nc.vector.tensor_tensor(masked, probs, m1.to_broadcast((128, TC, N_EXP)),
                                op=mybir.AluOpType.is_equal)
        nc.vector.tensor_scalar_mul(masked, masked, -10.0)
        nc.vector.tensor_add(masked, masked, probs)
        m2 = small_sb.tile([128, TC, 1], F32, tag="m2")
        nc.vector.reduce_max(m2, masked, axis=mybir.AxisListType.X)
        disp = small_sb.tile([128, TC, N_EXP], F32, tag="disp")
        nc.vector.tensor_tensor(disp, probs, m2.to_broadcast((128, TC, N_EXP)),
                                op=mybir.AluOpType.is_ge)
        nc.vector.tensor_mul(disp, disp, probs)
        # transpose each [128, 8] chunk -> [8, 128] into dispT
        for tci in range(TC):
            trp = ps512.tile([N_EXP, 128], F32, tag="ps512", name="trp")
            nc.tensor.transpose(trp, disp[:, tci, :], ident)
            nc.vector.tensor_copy(dispT[:, ts(tci, 128)], trp)

        # a_downT[nr, t] = (scaled A_flat) @ x  ;  cT = a_downT * expand(dispT)
        adn = ps512.tile([128, T], F32, tag="ps512")
        for k in range(DC):
            nc.tensor.matmul(adn, (aT[k]), (xT[k]),
                             start=(k == 0), stop=(k == DC - 1))
        adn_sb = acts.tile([128, T], F32, tag="adn_sb")
        nc.vector.tensor_copy(adn_sb, adn)
        expd = ps512.tile([128, T], F32, tag="ps512")
        nc.tensor.matmul(expd, (e8), (dispT), start=True, stop=True)
        cT = acts.tile([128, T], F32R, tag="cT")
        nc.vector.tensor_mul(cT, adn_sb, expd)

        # ---- v natural + ones columns: v_aug[tc][128, 12, 128] bf16 ----
        v_aug = []
        for tci in range(TC):
            pva = ps512.tile([128, 512], F32, tag="ps512", name="pva")
            pvb = ps512.tile([128, 256], F32, tag="ps512", name="pvb")
            for (pv, n0, nsz) in ((pva, 0, 512), (pvb, 512, 256)):
                for k in range(DC):
                    nc.tensor.matmul(pv[:, 0:nsz],
                                     (xT[k][:, ts(tci, 128)]),
                                     (wv[k][:, n0:n0 + nsz]),
                                     start=(k == 0), stop=False)
                nc.tensor.matmul(pv[:, 0:nsz], (cT[:, ts(tci, 128)]),
                                 (bv[:, n0:n0 + nsz]), start=False, stop=True)
            va = acts.tile([128, H, 2 * HD], BF16, tag=f"v_aug{tci}")
            nc.vector.tensor_add(va[:, 0:8, 0:HD],
                                 pva.rearrange("p (h c) -> p h c", c=HD),
                                 bvv_full[:, 0:512].rearrange(
                                     "p (h c) -> p h c", c=HD))
            nc.vector.tensor_add(va[:, 8:12, 0:HD],
                                 pvb.rearrange("p (h c) -> p h c", c=HD),
                                 bvv_full[:, 512:768].rearrange(
                                     "p (h c) -> p h c", c=HD))
            # 64 ones-columns: the O-matmul then lands Z on psum partitions
            # 64:128, so the softmax normalizer needs no cross-partition move
            nc.vector.memset(va[:, :, HD:2 * HD], 1.0)
            v_aug.append(va)

        # ---- fused qk-projection + attention, software-pipelined by head
        #      pair: emit pair j's qkT matmuls and score/exp stage, then pair
        #      j-1's output matmuls, so the PE never waits on the ACT exps ----
        qkT = [None] * QKC
        aoT = [acts.tile([128, T], F32R, tag=f"aoT{dc}", name=f"aoT{dc}")
               for dc in range(DC)]

        def emit_qk_pair(j):
            for oc in (j, 6 + j):
                pq = ps512.tile([128, T], F32, tag="ps512", name="pq")
                for k in range(DC):
                    nc.tensor.matmul(pq, (wqk[oc][:, ts(k, 128)]), (xT[k]),
                                     start=(k == 0), stop=False)
                nc.tensor.matmul(pq, (btqk[:, ts(oc, 128)]), (cT),
                                 start=False, stop=True)
                sb = acts.tile([128, T], BF16, tag=f"qkT{oc}", name=f"qkT{oc}")
                nc.scalar.activation(sb, pq,
                                     mybir.ActivationFunctionType.Identity,
                                     bias=bqk[:, oc:oc + 1])
                qkT[oc] = sb

        def emit_st(j):
            qt = qkT[j]
            kt = qkT[6 + j]
            st_exp = {0: [], 64: []}
            for kc in range(TC):
                for po in (0, 64):
                    pst = ps512.tile([128, T], F32, tag="ps512", name="pst")
                    nc.tensor.matmul(pst, (kt[po:po + HD, ts(kc, 128)]),
                                     (qt[po:po + HD, :]), start=True, stop=True,
                                     tile_position=(po, 0))
                    se = stx.tile([128, T], BF16, tag="st_exp", name="se")
                    nc.scalar.activation(se, pst,
                                         mybir.ActivationFunctionType.Exp)
                    st_exp[po].append(se)
            return st_exp

        def emit_ot(j, st_exp):
            for po in (0, 64):
                h = 2 * j + po // 64
                pot = ps512.tile([128, T], F32, tag="ps512", name="pot")
                for kc in range(TC):
                    nc.tensor.matmul(pot, v_aug[kc][:, h, :], st_exp[po][kc],
                                     start=(kc == 0), stop=(kc == TC - 1))
                # rows 64:128 of pot are 64 copies of Z[q]
                rzb = small_sb.tile([HD, T], F32, tag="rzb", name="rzb")
                nc.vector.reciprocal(rzb, pot[HD:2 * HD, :])
                nc.vector.tensor_mul(aoT[j][po:po + HD, :], pot[0:HD, :], rzb)

        emit_qk_pair(0)
        prev = (0, emit_st(0))
        for j in range(1, H // 2):
            emit_qk_pair(j)
            cur = (j, emit_st(j))
            emit_ot(*prev)
            prev = cur
        emit_ot(*prev)

        # ---- final projection ----
        for tci in range(TC):
            pfa = ps512.tile([128, 512], F32, tag="ps512", name="pfa")
            pfb = ps512.tile([128, 256], F32, tag="ps512", name="pfb")
            for (pf, n0, nsz) in ((pfa, 0, 512), (pfb, 512, 256)):
                for dc in range(DC):
                    nc.tensor.matmul(pf[:, 0:nsz],
                                     (aoT[dc][:, ts(tci, 128)]),
                                     (wp[dc][:, n0:n0 + nsz]),
                                     start=(dc == 0), stop=(dc == DC - 1))
            osb = acts.tile([128, D], F32, tag=f"out_sb{tci}")
            nc.vector.tensor_add(osb[:, 0:512], pfa, bp_full[:, 0:512])
            nc.vector.tensor_add(osb[:, 512:768], pfb, bp_full[:, 512:768])
            nc.sync.dma_start(out=out_d[ts(tci, 128), :], in_=osb)


def prep_inputs(x, W_qkv, b_qkv, W_gate, b_gate, A, B_lora, W_proj, b_proj):
    """Host-side prep: pre-transpose/pre-scale weights, shard x by batch."""
    scale = HD ** -0.5
    scaling = ALPHA / RANK
    W_qkv = np.asarray(W_qkv, np.float32).copy()
    b_qkv = np.asarray(b_qkv, np.float32).copy()
    B_lora = np.asarray(B_lora, np.float32).copy()
    W_qkv[:D] *= scale          # fold attention scale into q
    b_qkv[:D] *= scale
    B_lora[:, :D, :] *= scale

    wqkT = W_qkv[:2 * D].T                                      # [768, 1536]
    # o-chunk-tiled: wqk_tiled[oc, p, k*128+f] = wqkT[k*128+p, oc*128+f]
    wqk_tiled = np.ascontiguousarray(
        wqkT.reshape(DC, 128, QKC, 128).transpose(2, 1, 0, 3).reshape(
            QKC, 128, DC * 128))
    wvT = np.ascontiguousarray(W_qkv[2 * D:].T)                 # [768, 768]
    wgT = np.ascontiguousarray(np.asarray(W_gate, np.float32).T)  # [768, 8]
    aT = np.ascontiguousarray(
        (np.asarray(A, np.float32).reshape(NR, D) * scaling).T)  # [768, 128]
    bt = np.ascontiguousarray(
        B_lora.transpose(0, 2, 1).reshape(NR, O3))               # [128, 2304]
    btqk = np.ascontiguousarray(bt[:, :2 * D])
    bvm = np.ascontiguousarray(bt[:, 2 * D:])
    wpT = np.ascontiguousarray(np.asarray(W_proj, np.float32).T)
    bqk = np.ascontiguousarray(b_qkv[:2 * D].reshape(QKC, 128).T)  # [128, 12]
    bvv = np.ascontiguousarray(b_qkv[2 * D:].reshape(1, D))
    bg = np.ascontiguousarray(np.asarray(b_gate, np.float32).reshape(1, N_EXP))
    bp = np.ascontiguousarray(np.asarray(b_proj, np.float32).reshape(1, D))
    e8 = np.ascontiguousarray(np.repeat(np.eye(N_EXP, dtype=np.float32), RANK, axis=1))

    shared = dict(wqkT=wqk_tiled, wvT=wvT, wgT=wgT, aT=aT, btqk=btqk, bv=bvm,
                  wpT=wpT, bqk=bqk, bvv=bvv, bg=bg, bp=bp, e8=e8)
    x = np.asarray(x, np.float32)
    in_maps = []
    for b in range(N_CORES):
        m = dict(shared)
        m["xT"] = np.ascontiguousarray(x[b].T)
        in_maps.append(m)
    return in_maps


def _install_ntff_shim():
    """run_bass_kernel_spmd(trace=True) under axon needs antenv.axon_hooks."""
    if "antenv.axon_hooks" in sys.modules:
        return
    try:
        from trn_agent_boot.trn_boot import _ntff_profile_via_ctypes
        hook = _ntff_profile_via_ctypes("/opt/axon/libaxon_pjrt.so")
    except Exception:
        hook = None
    mod = types.ModuleType("antenv.axon_hooks")
    mod.get_axon_ntff_profile_hook = lambda: hook
    mod.set_axon_ntff_profile_hook = lambda h: None
    sys.modules["antenv.axon_hooks"] = mod


_NC_CACHE = None


def kernel(x, W_qkv, b_qkv, W_gate, b_gate, A, B_lora, W_proj, b_proj,
           _trace=False):
    global _NC_CACHE
    if _NC_CACHE is None:
        _NC_CACHE = build_nc()
    nc = _NC_CACHE
    in_maps = prep_inputs(x, W_qkv, b_qkv, W_gate, b_gate, A, B_lora,
                          W_proj, b_proj)
    if _trace:
        _install_ntff_shim()
    res = run_bass_kernel_spmd(nc, in_maps, list(range(N_CORES)), trace=_trace)
    out = np.stack([res.results[i]["out"] for i in range(N_CORES)], axis=0)
    out = out.reshape(B_SZ, S, D)
    if _trace:
        kernel.last_exec_time_ns = res.exec_time_ns
        kernel.last_results = res
    return out
